# revision 3
# baseline (speedup 1.0000x reference)
"""Trainium2 Bass kernel for AR1ScanTV — v9 (folded projection, o-split).

Math: reference computes
    ab = x @ W_ab;  a = tanh(ab[...,0]);  b = ab[...,1:]
    h_t = a_t * h_{t-1} + b_t   (a_t scalar per timestep, broadcast over H)
    out = h @ Wy

Because a_t is a *scalar* per (batch, t) and the scan is linear in b, the
scan commutes with the right-multiplication by Wy:
    scan(a, x @ Wb) @ Wy == scan(a, x @ (Wb @ Wy))
so the device runs ONE big matmul (x @ W2, W2 = Wb @ Wy precomputed on
host) followed by the same hardware scan — half the tensor-engine work of
the two-matmul formulation.

Sharding: 8 cores = 4 batches x 2 halves of the OUTPUT channels (the
scan is independent per channel, so each core runs the full T=4096
recurrence for its 512 channels). No inter-core dependency at all: no
collective, no warmup redundancy — each core's 131072 matmul rows are
the exact MAC minimum for its half.

a_raw rides along as channel 0 of each core's W2 half (that column is
replaced by wa on device); the two sacrificed true output channels
(global 0 and 512) are recomputed exactly on host (two 1024-dim
projections + a 4096-step scalar scan).

Outputs leave the device transposed ([channel, t]); host transposes.
"""

import numpy as np

B, T, D = 4, 4096, 1024
DH = D // 2              # output channels per core
NCORES = 8
NO = DH // 128           # output-channel partition tiles per core (4)
NK = D // 128            # contraction partition tiles (8)
NB = T // 512            # 512-step time blocks (8)

_CACHE = {}
KVER = "v10a"  # bump on every kernel change
NDUM = 75     # 64-row warmup matmuls (see p-state comment below)


def _build_program(num_devices: int = NCORES, suffix: str = ""):
    from contextlib import ExitStack

    import concourse.bass as bass
    import concourse.mybir as mybir
    import concourse.tile as tile
    from concourse import bacc

    f32 = mybir.dt.float32
    bf16 = mybir.dt.bfloat16
    AF = mybir.ActivationFunctionType
    ALU = mybir.AluOpType

    nc = bacc.Bacc(
        "TRN2",
        target_bir_lowering=False,
        debug=False,
        enable_asserts=False,
        num_devices=num_devices,
    )

    # tensor names carry a build tag: the axon-side executable cache keys on
    # the HLO signature only (not the embedded bass program), so distinct
    # builds must have distinct tensor names to avoid stale-NEFF collisions.
    tag = f"{KVER}{suffix}x{num_devices}"
    xT_d = nc.dram_tensor(f"xT_{tag}", [D, T], bf16, kind="ExternalInput").ap()
    # W2 half, rows pre-shuffled on host: row no*128+k holds the (no,k)
    # weights laid out as (nk, oc) so each o-tile loads with 2KB-contiguous
    # descriptors.
    W2_d = nc.dram_tensor(f"W2_{tag}", [DH, D], bf16, kind="ExternalInput").ap()
    # out leaves the device in bf16 (host upcasts): halves out-DMA traffic
    # and the end-of-program transfer on the critical tail. The scan keeps
    # fp32 state internally; only the stored values round to bf16.
    out_d = nc.dram_tensor(f"out_{tag}", [DH, T], bf16, kind="ExternalOutput").ap()
    nc._ar1_tag = tag

    with tile.TileContext(nc) as tc, ExitStack() as ctx:
        xpool = ctx.enter_context(tc.tile_pool(name="xpool", bufs=3))
        wpool = ctx.enter_context(tc.tile_pool(name="wpool", bufs=1))
        misc = ctx.enter_context(tc.tile_pool(name="misc", bufs=1))
        abc = ctx.enter_context(tc.tile_pool(name="abc", bufs=2))
        pp = ctx.enter_context(tc.tile_pool(name="pp", bufs=6, space="PSUM"))

        xview = xT_d.rearrange("(nk k) t -> k nk t", k=128)
        outview = out_d.rearrange("(no o) t -> o no t", o=128)

        # W2 in SBUF: [k, no, nk, oc] so lhsT for (no, nk) is a plain slice.
        W2_s = wpool.tile([128, NO, NK, 128], bf16, tag="w2")
        out_s = misc.tile([128, NO, T], bf16, tag="out")
        a_row = misc.tile([1, T], f32, tag="a_row")

        def load_w2(no):
            nc.sync.dma_start(
                out=W2_s[:, no, :, :],
                in_=W2_d[no * 128:(no + 1) * 128, :].rearrange(
                    "k (nk oc) -> k nk oc", oc=128),
            )

        # Startup DMA order is latency-tuned: each DMA instruction costs
        # ~625ns of in-order HWDGE descriptor generation before its
        # transfer, so the first matmul's critical path is W2[o-tile 0] +
        # the first half of x block 0. The other W2 o-tiles stream behind,
        # each arriving just before the block-0 matmuls reach them.
        load_w2(0)
        xtiles = [xpool.tile([128, NK, 512], bf16, tag="xblk", name="xblk0")]
        nc.sync.dma_start(out=xtiles[0][:, 0:2, :], in_=xview[:, 0:2, 0:512])
        nc.sync.dma_start(out=xtiles[0][:, 2:4, :], in_=xview[:, 2:4, 0:512])
        load_w2(1)
        nc.sync.dma_start(out=xtiles[0][:, 4:6, :], in_=xview[:, 4:6, 0:512])
        nc.sync.dma_start(out=xtiles[0][:, 6:8, :], in_=xview[:, 6:8, 0:512])
        for no in range(2, NO):
            load_w2(no)

        # PE p-state warmup: the cost model ramps the tensor clock
        # 0.65->1.2->2.4 GHz over 3us of continuous busy. Burn the
        # unavoidable startup-DMA window (first weights + x block) with
        # dummy matmuls so the real stream starts at full clock. The feed
        # tile is only [128,64] so its memset (127ns on DVE) gates the
        # first dummy ~1us earlier than a full-block memset would; 64-row
        # dummies then keep PE busy until the first x chunk's semaphore.
        # NDUM is sized so the dummy stream ends just AFTER data-ready —
        # ending early would idle PE and reset the p-state ramp.
        ppd = ctx.enter_context(tc.tile_pool(name="ppd", bufs=1, space="PSUM"))
        dum = misc.tile([128, 64], bf16, tag="dum", name="dummy_in")
        nc.vector.memset(dum[:, :], 0.0)
        pd = ppd.tile([64, 64], f32, tag="pd", name="dummy_ps")
        for i in range(NDUM):
            nc.tensor.matmul(pd[:, :], dum[:, :], dum[:, :],
                             start=(i == 0), stop=(i == NDUM - 1))

        segs = [(tb * 512, 512) for tb in range(NB)]

        for off, S in segs:
            sl = slice(off, off + S)
            tb = off // 512
            xblk = xtiles[tb]
            xsl = slice(off % 512, off % 512 + S)
            # prefetch next block's x ahead of this block's out-DMAs
            # (HWDGE queue is in-order)
            if off % 512 == 0 and tb + 1 < NB:
                nxt = xpool.tile([128, NK, 512], bf16, tag="xblk",
                                 name=f"xblk{tb + 1}")
                nc.sync.dma_start(
                    out=nxt[:, :, :],
                    in_=xview[:, :, off + 512:off + 1024])
                xtiles.append(nxt)

            a_bc = abc.tile([128, 512], f32, tag="a_bc")
            # The very last o-tile of the last block is split 256+256 so
            # the end-of-program scan -> out-DMA -> sem -> drain chain
            # rides on a half-size chunk (256-row matmuls still beat the
            # 71ns PE SEQ decode, so no decode bubbles).
            osegs = [(no, sl, slice(off % 512, off % 512 + S))
                     for no in range(NO)]
            if off == (NB - 1) * 512:
                osegs = osegs[:-1] + [
                    (NO - 1, slice(off, off + 256), slice(0, 256)),
                    (NO - 1, slice(off + 256, off + 512), slice(256, 512))]
            for no, sl, xsl in osegs:
                S = sl.stop - sl.start
                pj = pp.tile([128, 512], f32, tag="pj")
                for nk in range(NK):
                    nc.tensor.matmul(
                        pj[:, :S],
                        W2_s[:, no, nk, :],
                        xblk[:, nk, xsl],
                        start=(nk == 0),
                        stop=(nk == NK - 1),
                    )
                if no == 0:
                    # channel 0 of o-tile 0 is a_raw (W2-half col 0 == wa)
                    nc.scalar.activation(a_row[0:1, sl], pj[0:1, :S], AF.Tanh)
                    nc.gpsimd.partition_broadcast(a_bc[:, :S], a_row[0:1, sl])
                # h_t = a_t * h_{t-1} + b_t, chained across blocks in place
                nc.vector.tensor_tensor_scan(
                    out_s[:, no, sl], a_bc[:, xsl.start:xsl.start + S],
                    pj[:, :S],
                    0.0 if sl.start == 0 else out_s[:, no, sl.start - 1:sl.start],
                    ALU.mult, ALU.add,
                )
                nc.sync.dma_start(out=outview[:, no, sl], in_=out_s[:, no, sl])
    nc.compile()
    return nc


def _get_program(suffix: str = ""):
    key = ("prog", suffix)
    if key not in _CACHE:
        _CACHE[key] = _build_program(suffix=suffix)
    return _CACHE[key]


def _prep_weights(W_ab, Wy):
    """W2 = Wb @ Wy split into o-halves, column 0 of each half swapped for
    wa, rows pre-shuffled into the [no*128+k, (nk, oc)] order the device
    DMA expects. Returns ([Z0, Z1], wa, sac_cols) — wa and the two
    sacrificed true columns feed the host-side channel reconstruction."""
    import ml_dtypes

    wa = np.ascontiguousarray(W_ab[:, 0], dtype=np.float32)
    Wb = np.ascontiguousarray(W_ab[:, 1:], dtype=np.float32)
    W2 = Wb @ np.asarray(Wy, dtype=np.float32)
    sac_cols = W2[:, [0, DH]].copy()
    Zs = []
    for p in range(2):
        half = W2[:, p * DH:(p + 1) * DH].copy()
        half[:, 0] = wa
        Zs.append(np.ascontiguousarray(
            half.reshape(NK, 128, NO, 128).transpose(2, 1, 0, 3).reshape(DH, D)
        ).astype(ml_dtypes.bfloat16))
    return Zs, wa, sac_cols


def _make_in_maps(x, Zs, tag):
    import ml_dtypes

    xTs = [np.ascontiguousarray(x[b].T.astype(ml_dtypes.bfloat16))
           for b in range(B)]
    in_maps = []
    for core in range(NCORES):
        b, p = core // 2, core % 2
        in_maps.append({f"xT_{tag}": xTs[b], f"W2_{tag}": Zs[p]})
    return in_maps


def _host_channels(x, wa, sac_cols):
    """Exact (fp64 scan) recomputation of the two sacrificed output
    channels (global 0 and DH) for all batches."""
    xf = x.reshape(-1, D)
    a = np.tanh((xf @ wa).reshape(B, T).astype(np.float64))
    bb = (xf @ sac_cols).reshape(B, T, 2).astype(np.float64)
    out01 = np.empty((B, T, 2), dtype=np.float64)
    h = np.zeros((B, 2), dtype=np.float64)
    for t in range(T):
        h = a[:, t:t + 1] * h + bb[:, t]
        out01[:, t] = h
    return out01.astype(np.float32)


def kernel(x, W_ab, b_ab, Wy, by, _collect_results=None, **run_kwargs):
    """Full-input / full-output entry point. b_ab/by are zeros by spec."""
    from concourse.bass_utils import run_bass_kernel_spmd

    x = np.asarray(x, dtype=np.float32)
    Zs, wa, sac_cols = _prep_weights(np.asarray(W_ab, dtype=np.float32), Wy)

    nc = _get_program()
    tag = nc._ar1_tag
    in_maps = _make_in_maps(x, Zs, tag)
    try:
        res = run_bass_kernel_spmd(
            nc, in_maps, core_ids=list(range(NCORES)), **run_kwargs)
    except Exception:
        # One retry: the axon-proxied runtime occasionally reports a
        # transient device error (NRT_EXEC_UNIT_UNRECOVERABLE) on a
        # first execution. Rebuild under a fresh tensor-name tag so the
        # executable cache cannot serve the same NEFF, and rerun.
        nc = _get_program(suffix="r")
        tag = nc._ar1_tag
        in_maps = _make_in_maps(x, Zs, tag)
        res = run_bass_kernel_spmd(
            nc, in_maps, core_ids=list(range(NCORES)), **run_kwargs)

    out = np.empty((B, T, D), dtype=np.float32)
    for core in range(NCORES):
        b, p = core // 2, core % 2
        shard = res.results[core][f"out_{tag}"]  # [DH, T] bf16, channel-major
        out[b, :, p * DH:(p + 1) * DH] = shard.T.astype(np.float32)
    out[:, :, [0, DH]] = _host_channels(x, wa, sac_cols)
    if _collect_results is not None:
        _collect_results.append(res)
    return out

